# revision 1
# baseline (speedup 1.0000x reference)
"""Trainium2 Bass kernel for Gemma4Audio chunked local attention.

Sharding: 8 cores = batch(4) x seq-half(2). Each core processes 3072
tokens of one batch (plus a 12-token left halo for k/v context) fully
locally -- the block-local attention never crosses the half-sequence
boundary mid-block, so no collectives are needed.

Device algorithm per core (all in transposed [feature, token] layout):
  - q/k/v projections as W.T-chunk @ xT matmuls (fp32r, N=384/396)
  - per 96-query tile: scores = qT.T @ kT_window (fp32), rel-position
    scores via two small matmuls (normal q and q shifted one token left
    for the rel_shift "row leak" term), placed diagonally into window
    coords by one GPSIMD local_scatter with per-partition index tables
  - band mask add, softcap tanh, exp (no max-sub needed: |logits|<=50),
    row-sum via ACT accum_out, normalize, PE-transpose attn and V,
    out.T = V_tok.T @ attn.T, post projection back to [1024, tokens].
"""

import math

import numpy as np

# ---- model constants (hardcoded per problem spec) ----
HID = 1024
H = 8
D = 128
CHUNK = 12
PAST = 12
CTX = 24  # context window per block
P = 25  # relative positions
SOFTCAP = 50.0
Q_SCALE = D ** (-0.5) / math.log(2)
K_SCALE = math.log(1.0 + math.e) / math.log(2)

B = 4
S = 6144
NCORES = 8

T = S // 2  # 3072 tokens per core
THALO = T + PAST  # 3084 with left halo
TR = 384  # tokens per region
NREG = T // TR  # 8
TPB = 96  # queries per attention tile (8 blocks)
NTILE = TR // TPB  # 4
W = TPB + CHUNK  # 108-key window per tile
KC = HID // 128  # 8 contraction chunks
NP25 = 26  # padded rel-position count for scatter (even)
DFREE = 112  # scatter destination free size (>= W, even)

_CACHE = {}


def _build_tables():
    """Host-precomputed scatter index tables and band mask (batched over
    the NTILE tiles of a head-region: targets offset by g*DFREE)."""
    idx = np.full((128, NTILE * 2 * NP25), -1, dtype=np.int16)
    mask = np.full((128, NTILE * DFREE), -1e9, dtype=np.float32)  # cast to bf16 at ship
    for a in range(TPB):
        i, c = divmod(a, CHUNK)
        for g in range(NTILE):
            mask[a, g * DFREE + 12 * i : g * DFREE + 12 * i + CTX] = 0.0
            for p in range(P):
                # term A: own-row rel score at window col a+p (ctx col c+p)
                if c + p < CTX:
                    idx[a, g * 2 * NP25 + p] = g * DFREE + a + p
                # term B (rel_shift row leak): prev query's rel score
                if p >= P - c:
                    idx[a, g * 2 * NP25 + NP25 + p] = g * DFREE + a + p - P
    return idx, mask


def _build_bass():
    import concourse.bass as bass
    import concourse.bacc as bacc
    import concourse.mybir as mybir
    import concourse.tile as tile
    from concourse.masks import make_identity

    dt = mybir.dt
    f32 = dt.float32
    f32r = dt.float32r
    bf16 = dt.bfloat16
    AF = mybir.ActivationFunctionType

    nc = bacc.Bacc(None, target_bir_lowering=False)

    bf16_d = dt.bfloat16
    xT = nc.declare_dram_parameter("xT", [HID, THALO], f32r, isOutput=False)
    wqT = nc.declare_dram_parameter("wqT", [HID, HID], f32r, isOutput=False)
    wkT = nc.declare_dram_parameter("wkT", [HID, HID], f32r, isOutput=False)
    wvT = nc.declare_dram_parameter("wvT", [HID, HID], f32r, isOutput=False)
    wpT = nc.declare_dram_parameter("wpT", [HID, HID], f32r, isOutput=False)
    wrelT = nc.declare_dram_parameter("wrelT", [HID, HID], f32, isOutput=False)
    pembT = nc.declare_dram_parameter("pembT", [HID, 32], f32, isOutput=False)
    idxtab = nc.declare_dram_parameter("idxtab", [128, NTILE * 2 * NP25], dt.int16, isOutput=False)
    masktab = nc.declare_dram_parameter("masktab", [128, NTILE * DFREE], bf16_d, isOutput=False)
    outT = nc.declare_dram_parameter("outT", [HID, T], f32, isOutput=True)

    def r32(ap):
        return ap.bitcast(f32r)

    with tile.TileContext(nc) as tc:
        with (
            tc.tile_pool(name="consts", bufs=1) as cpool,
            tc.tile_pool(name="psum", bufs=3, space="PSUM") as pspool,
            tc.tile_pool(name="psumS", bufs=1, space="PSUM") as pspoolS,
            tc.tile_pool(name="psumP", bufs=3, space="PSUM") as pspoolP,
        ):
            idx_sb = cpool.tile([128, NTILE * 2 * NP25], dt.int16, tag="idx")
            nc.sync.dma_start(out=idx_sb[:], in_=idxtab[:, :])
            mask_sb = cpool.tile([128, NTILE * DFREE], bf16, tag="mask")
            nc.sync.dma_start(out=mask_sb[:], in_=masktab[:, :])
            ident = cpool.tile([128, 128], f32, tag="ident")
            make_identity(nc, ident[:])

            # ---- rel_k = Wrel @ pos_emb.T per head: [128, 32] slices ----
            relk_sb = cpool.tile([128, H, 32], f32r, tag="relk")
            with tc.tile_pool(name="wrelp", bufs=2) as wrelpool:
                pemb_sb = wrelpool.tile([128, KC, 32], f32, tag="pemb")
                nc.sync.dma_start(
                    out=pemb_sb[:], in_=pembT.rearrange("(kc p) o -> p kc o", p=128)
                )
                for h in range(H):
                    wrel_sb = wrelpool.tile([128, KC, 128], f32, tag="wrel")
                    nc.sync.dma_start(
                        out=wrel_sb[:],
                        in_=wrelT[:, h * 128 : (h + 1) * 128].rearrange(
                            "(kc p) o -> p kc o", p=128
                        ),
                    )
                    ps = pspool.tile([128, 32], f32, tag="ps")
                    for kc in range(KC):
                        nc.tensor.matmul(
                            ps[:],
                            lhsT=wrel_sb[:, kc, :],
                            rhs=pemb_sb[:, kc, :],
                            start=(kc == 0),
                            stop=(kc == KC - 1),
                        )
                    nc.vector.tensor_copy(relk_sb[:, h, :], ps[:])

            with (
                tc.tile_pool(name="weights", bufs=1) as wpool,
                tc.tile_pool(name="xin", bufs=2) as xpool,
                tc.tile_pool(name="strips", bufs=3) as spool,
                tc.tile_pool(name="ao", bufs=1) as aopool,
                tc.tile_pool(name="attn", bufs=3) as apool,
                tc.tile_pool(name="outsb", bufs=2) as opool,
            ):
                w_sb = {}
                for name, drh in (("wq", wqT), ("wk", wkT), ("wv", wvT), ("wp", wpT)):
                    t = wpool.tile([128, KC, HID], f32r, tag=name)
                    nc.sync.dma_start(
                        out=t[:], in_=drh.rearrange("(kc p) o -> p kc o", p=128)
                    )
                    w_sb[name] = t
                _main(nc, tc, mybir, AF, w_sb, xT, outT, idx_sb, mask_sb, ident,
                      relk_sb, xpool, spool, aopool, apool, opool, pspool,
                      pspoolP, pspoolS, f32, f32r, bf16)
    nc.compile()
    return nc


def _main(nc, tc, mybir, AF, w_sb, xT, outT, idx_sb, mask_sb, ident, relk_sb,
          xpool, spool, aopool, apool, opool, pspool, pspoolP, pspoolS, f32, f32r, bf16):
            # ---- main loop over token regions ----
            for r in range(NREG):
                xr = xpool.tile([128, KC, TR + PAST], f32r, tag="xr")
                xsrc = xT[:, r * TR : r * TR + TR + PAST].rearrange(
                    "(kc p) n -> p kc n", p=128
                )
                for kc in range(KC):
                    nc.sync.dma_start(out=xr[:, kc, :], in_=xsrc[:, kc, :])
                aoT = aopool.tile([128, H * TR], f32r, tag="aoT")

                for h in range(H):
                    hs = slice(h * 128, (h + 1) * 128)
                    # --- projections (fp32r, full PE rate) ---
                    qps = pspoolP.tile([128, TR + 2], f32, tag="pj")
                    kps = pspoolP.tile([128, TR + PAST], f32, tag="pj")
                    vps = pspoolP.tile([128, TR + PAST], f32, tag="pj")
                    for kc in range(KC):
                        st, sp = kc == 0, kc == KC - 1
                        nc.tensor.matmul(
                            qps[:],
                            lhsT=w_sb["wq"][:, kc, hs],
                            rhs=xr[:, kc, PAST - 2 :],
                            start=st,
                            stop=sp,
                        )
                        nc.tensor.matmul(
                            kps[:],
                            lhsT=w_sb["wk"][:, kc, hs],
                            rhs=xr[:, kc, :],
                            start=st,
                            stop=sp,
                        )
                        nc.tensor.matmul(
                            vps[:],
                            lhsT=w_sb["wv"][:, kc, hs],
                            rhs=xr[:, kc, :],
                            start=st,
                            stop=sp,
                        )
                    q_sb = spool.tile([128, TR + 2], f32r, tag="q")
                    k_sb = spool.tile([128, 544], f32r, tag="k")
                    v_sb = spool.tile([128, TR + PAST], f32, tag="v")
                    nc.vector.tensor_copy(q_sb[:], qps[:])
                    nc.scalar.copy(k_sb[:, 0 : TR + PAST], kps[:])
                    nc.vector.memset(k_sb[:, TR + PAST : 544].bitcast(f32), 0.0)
                    nc.scalar.copy(v_sb[:], vps[:])

                    # --- attention: all NTILE tiles of the region batched ---
                    sall = pspoolS.tile([TPB, NTILE, 256], f32, tag="sall")
                    bdall = pspool.tile([TPB, NTILE, 64], f32, tag="ps")
                    for g in range(NTILE):
                        b0 = TPB * g
                        qmain = q_sb[:, b0 + 2 : b0 + 2 + TPB]
                        qprev = q_sb[:, b0 + 1 : b0 + 1 + TPB]
                        nc.tensor.matmul(
                            sall[:, g, :], lhsT=qmain,
                            rhs=k_sb[:, b0 : b0 + 256], start=True, stop=True,
                        )
                        nc.tensor.matmul(
                            bdall[:, g, 0:NP25], lhsT=qmain,
                            rhs=relk_sb[:, h, 0:NP25], start=True, stop=True,
                        )
                        nc.tensor.matmul(
                            bdall[:, g, 32 : 32 + NP25], lhsT=qprev,
                            rhs=relk_sb[:, h, 0:NP25], start=True, stop=True,
                        )

                    data = apool.tile([TPB, NTILE, 2 * NP25], bf16, tag="data")
                    nc.vector.tensor_copy(data[:, :, 0:NP25], bdall[:, :, 0:NP25])
                    nc.vector.tensor_copy(data[:, :, NP25:], bdall[:, :, 32 : 32 + NP25])
                    dst = apool.tile([TPB, NTILE * DFREE], bf16, tag="dst")
                    nc.gpsimd.local_scatter(
                        dst[:], data[:], idx_sb[0:TPB, :],
                        channels=TPB, num_elems=NTILE * DFREE,
                        num_idxs=NTILE * 2 * NP25,
                    )

                    nc.vector.tensor_tensor(
                        out=dst[:], in0=dst[:], in1=mask_sb[0:TPB, :],
                        op=mybir.AluOpType.add,
                    )
                    lg = apool.tile([TPB, NTILE, DFREE], f32, tag="lg")
                    nc.vector.tensor_tensor(
                        out=lg[:], in0=sall[:, :, 0:DFREE], in1=dst.rearrange("p (g w) -> p g w", g=NTILE),
                        op=mybir.AluOpType.add,
                    )
                    nc.scalar.activation(
                        out=lg[:], in_=lg[:], func=AF.Tanh, scale=1.0 / SOFTCAP
                    )
                    nc.scalar.activation(
                        out=lg[:], in_=lg[:], func=AF.Exp, scale=SOFTCAP
                    )
                    rsum = apool.tile([TPB, NTILE], f32, tag="rsum")
                    nc.vector.tensor_reduce(
                        out=rsum[:], in_=lg[:], axis=mybir.AxisListType.X,
                        op=mybir.AluOpType.add,
                    )
                    nc.vector.reciprocal(rsum[:], rsum[:])

                    atps = pspool.tile([W, NTILE, TPB], f32, tag="ps")
                    vtps = pspool.tile([W, NTILE, 128], f32, tag="ps")
                    aops = pspoolS.tile([128, NTILE, 256], f32, tag="sall")
                    for g in range(NTILE):
                        nc.vector.tensor_scalar_mul(
                            out=lg[:, g, 0:W], in0=lg[:, g, 0:W],
                            scalar1=rsum[:, g : g + 1],
                        )
                    at_sb = apool.tile([W, 544], f32r, tag="at")
                    vt_sb = apool.tile([W, NTILE, 128], f32r, tag="vt")
                    for g in range(NTILE):
                        b0 = TPB * g
                        nc.tensor.transpose(
                            atps[:, g, :], lg[:, g, 0:W], ident[0:TPB, 0:TPB]
                        )
                        nc.tensor.transpose(
                            vtps[:, g, :], v_sb[:, b0 : b0 + W], ident[:, :]
                        )
                    nc.vector.tensor_copy(at_sb[:, 0 : NTILE * TPB], atps[:])
                    nc.vector.memset(at_sb[:, NTILE * TPB : 544].bitcast(f32), 0.0)
                    nc.scalar.copy(vt_sb[:], vtps[:])
                    for g in range(NTILE):
                        nc.tensor.matmul(
                            aops[:, g, :], lhsT=vt_sb[:, g, :],
                            rhs=at_sb[:, TPB * g : TPB * g + 256],
                            start=True, stop=True,
                        )
                    nc.vector.tensor_copy(
                        aoT[:, h * TR : (h + 1) * TR], aops[:, :, 0:TPB]
                    )

                # --- post projection for this region ---
                for oc in range(KC):
                    pps = pspool.tile([128, TR], f32, tag="ps")
                    for h in range(H):
                        nc.tensor.matmul(
                            pps[:],
                            lhsT=w_sb["wp"][:, h, oc * 128 : (oc + 1) * 128],
                            rhs=aoT[:, h * TR : (h + 1) * TR],
                            start=(h == 0),
                            stop=(h == H - 1),
                        )
                    po = opool.tile([128, TR], f32, tag="po")
                    if oc % 2 == 0:
                        nc.vector.tensor_copy(po[:], pps[:])
                    else:
                        nc.scalar.copy(po[:], pps[:])
                    nc.sync.dma_start(
                        out=outT[oc * 128 : (oc + 1) * 128, r * TR : (r + 1) * TR],
                        in_=po[:],
                    )


def _get_nc():
    if "nc" not in _CACHE:
        _CACHE["nc"] = _build_bass()
    return _CACHE["nc"]


def _prepare_in_maps(hidden_states, position_embeddings, Wq, Wk, Wv, Wpost, Wrel,
                     per_dim_scale):
    f32 = np.float32
    hs = np.asarray(hidden_states, f32)
    pe = np.asarray(position_embeddings, f32)
    qscale = (Q_SCALE * np.log1p(np.exp(np.asarray(per_dim_scale, np.float64)))).astype(
        np.float64
    )
    qs_tiled = np.tile(qscale, H)  # per output channel o: scale[o % 128]
    wqT = np.ascontiguousarray((np.asarray(Wq, np.float64) * qs_tiled[:, None]).T.astype(f32))
    wkT = np.ascontiguousarray((np.asarray(Wk, np.float64) * K_SCALE).T.astype(f32))
    wvT = np.ascontiguousarray(np.asarray(Wv, f32).T)
    wpT = np.ascontiguousarray(np.asarray(Wpost, f32).T)
    wrelT = np.ascontiguousarray(np.asarray(Wrel, f32).T)
    pembT = np.zeros((HID, 32), f32)
    pembT[:, :P] = pe.T
    idx, mask = _build_tables()
    import ml_dtypes
    mask = mask.astype(ml_dtypes.bfloat16)

    shared = dict(wqT=wqT, wkT=wkT, wvT=wvT, wpT=wpT, wrelT=wrelT, pembT=pembT,
                  idxtab=idx, masktab=mask)
    in_maps = []
    for core in range(NCORES):
        b, half = divmod(core, 2)
        lo = half * T
        slab = np.zeros((THALO, HID), f32)
        src_lo = max(lo - PAST, 0)
        slab[PAST - (lo - src_lo) :, :] = hs[b, src_lo : lo + T, :]
        xT = np.ascontiguousarray(slab.T)
        in_maps.append(dict(xT=xT, **shared))
    return in_maps


def _assemble(results):
    out = np.empty((B, S, HID), np.float32)
    for core in range(NCORES):
        b, half = divmod(core, 2)
        out[b, half * T : (half + 1) * T, :] = results[core]["outT"].T
    return out


def kernel(**inputs) -> np.ndarray:
    from concourse.bass_utils import run_bass_kernel_spmd

    nc = _get_nc()
    in_maps = _prepare_in_maps(**inputs)
    res = run_bass_kernel_spmd(nc, in_maps, list(range(NCORES)))
    return _assemble(res.results)



# revision 4
# speedup vs baseline: 1.2384x; 1.2384x over previous
"""Trainium2 Bass kernel for Gemma4Audio chunked local attention.

Sharding: 8 cores = batch(4) x seq-half(2). Each core processes 3072
tokens of one batch (plus a 12-token left halo and 4-token right pad)
fully locally -- block-local attention never crosses the half-sequence
boundary mid-block, so no collectives are needed.

v2: fp16 operand pipeline (fp32 PSUM accumulation throughout), narrow
fp16 attention matmuls (112-wide scores, 96-wide outputs, fp16 PE
transposes), two-iteration software pipelining so the softmax chain
(scatter/mask/softcap/exp/normalize on DVE+ACT+GPSIMD) overlaps the
next heads' projection matmuls on the PE, GPSIMD offload for
scatter/mask/normalize, and direct PSUM->DRAM DMA for the post
projection output.
"""

import math

import numpy as np

# ---- model constants (hardcoded per problem spec) ----
HID = 1024
H = 8
D = 128
CHUNK = 12
PAST = 12
CTX = 24  # context window per block
P = 25  # relative positions
SOFTCAP = 50.0
Q_SCALE = D ** (-0.5) / math.log(2)
K_SCALE = math.log(1.0 + math.e) / math.log(2)

B = 4
S = 6144
NCORES = 8

T = S // 2  # 3072 tokens per core
THALO = T + PAST + 4  # 3088 with left halo + right pad for 112-wide windows
TR = 384  # tokens per region
NREG = T // TR  # 8
TPB = 96  # queries per attention tile (8 blocks)
NTILE = TR // TPB  # 4
WIN = 112  # key window per tile (96 + 12 band + 4 pad, masked)
W = 108  # live key columns feeding the output matmul
KC = HID // 128  # 8 contraction chunks
NP25 = 26  # padded rel-position count for scatter (even)
DFREE = 112  # scatter destination free size per tile
MASKVAL = -30000.0  # fits fp16; tanh saturates -> exp(-50) ~ 0

_CACHE = {}


def _build_tables():
    """Host-precomputed scatter index tables and band mask (batched over
    the NTILE tiles of a head-region: targets offset by g*DFREE)."""
    idx = np.full((128, NTILE * 2 * NP25), -1, dtype=np.int16)
    mask = np.full((128, NTILE * DFREE), MASKVAL, dtype=np.float16)
    for a in range(TPB):
        i, c = divmod(a, CHUNK)
        for g in range(NTILE):
            mask[a, g * DFREE + 12 * i : g * DFREE + 12 * i + CTX] = 0.0
            for p in range(P):
                # term A: own-row rel score at window col a+p (ctx col c+p)
                if c + p < CTX:
                    idx[a, g * 2 * NP25 + p] = g * DFREE + a + p
                # term B (rel_shift row leak): prev query's rel score
                if p >= P - c:
                    idx[a, g * 2 * NP25 + NP25 + p] = g * DFREE + a + p - P
    return idx, mask


def _build_bass():
    import concourse.bass as bass
    import concourse.bacc as bacc
    import concourse.mybir as mybir
    import concourse.tile as tile
    from concourse.masks import make_identity

    dt = mybir.dt
    f32 = dt.float32
    f16 = dt.float16
    AF = mybir.ActivationFunctionType
    ADD = mybir.AluOpType.add
    AXX = mybir.AxisListType.X

    nc = bacc.Bacc(None, target_bir_lowering=False)

    xT = nc.declare_dram_parameter("xT", [HID, THALO], f16, isOutput=False)
    wqT = nc.declare_dram_parameter("wqT", [HID, HID], f16, isOutput=False)
    wkT = nc.declare_dram_parameter("wkT", [HID, HID], f16, isOutput=False)
    wvT = nc.declare_dram_parameter("wvT", [HID, HID], f16, isOutput=False)
    wpT = nc.declare_dram_parameter("wpT", [HID, HID], f16, isOutput=False)
    wrelT = nc.declare_dram_parameter("wrelT", [HID, HID], f16, isOutput=False)
    pembT = nc.declare_dram_parameter("pembT", [HID, 32], f16, isOutput=False)
    idxtab = nc.declare_dram_parameter("idxtab", [128, NTILE * 2 * NP25], dt.int16, isOutput=False)
    masktab = nc.declare_dram_parameter("masktab", [128, NTILE * DFREE], f16, isOutput=False)
    outT = nc.declare_dram_parameter("outT", [HID, T], f32, isOutput=True)

    with tile.TileContext(nc) as tc:
        with (
            tc.tile_pool(name="consts", bufs=1) as cpool,
            tc.tile_pool(name="pj", bufs=3, space="PSUM") as pjpool,
            tc.tile_pool(name="psS", bufs=1, space="PSUM") as pspoolS,
            tc.tile_pool(name="psB", bufs=1, space="PSUM") as pspoolB,
            tc.tile_pool(name="psT", bufs=2, space="PSUM") as pspoolT,
            tc.tile_pool(name="psO", bufs=1, space="PSUM") as pspoolO,
        ):
            idx_sb = cpool.tile([128, NTILE * 2 * NP25], dt.int16, tag="idx")
            nc.sync.dma_start(out=idx_sb[:], in_=idxtab[:, :])
            mask_sb = cpool.tile([128, NTILE * DFREE], f16, tag="mask")
            nc.sync.dma_start(out=mask_sb[:], in_=masktab[:, :])
            ident = cpool.tile([128, 128], f16, tag="ident")
            make_identity(nc, ident[:])

            # ---- rel_k = Wrel @ pos_emb.T per head: [128, 32] slices ----
            relk_sb = cpool.tile([128, H, 32], f16, tag="relk")
            with tc.tile_pool(name="wrelp", bufs=2) as wrelpool:
                pemb_sb = wrelpool.tile([128, KC, 32], f16, tag="pemb")
                nc.sync.dma_start(
                    out=pemb_sb[:], in_=pembT.rearrange("(kc p) o -> p kc o", p=128)
                )
                for h in range(H):
                    wrel_sb = wrelpool.tile([128, KC, 128], f16, tag="wrel")
                    nc.sync.dma_start(
                        out=wrel_sb[:],
                        in_=wrelT[:, h * 128 : (h + 1) * 128].rearrange(
                            "(kc p) o -> p kc o", p=128
                        ),
                    )
                    ps = pjpool.tile([128, 32], f32, tag="pj")
                    for kc in range(KC):
                        nc.tensor.matmul(
                            ps[:],
                            lhsT=wrel_sb[:, kc, :],
                            rhs=pemb_sb[:, kc, :],
                            start=(kc == 0),
                            stop=(kc == KC - 1),
                        )
                    nc.vector.tensor_copy(relk_sb[:, h, :], ps[:])

            with (
                tc.tile_pool(name="weights", bufs=1) as wpool,
                tc.tile_pool(name="xin", bufs=2) as xpool,
                tc.tile_pool(name="strips", bufs=2) as spool,
                tc.tile_pool(name="ao", bufs=2) as aopool,
                tc.tile_pool(name="attn", bufs=2) as apool,
            ):
                w_sb = {}
                for name, drh in (("wq", wqT), ("wk", wkT), ("wv", wvT), ("wp", wpT)):
                    t = wpool.tile([128, KC, HID], f16, tag=name)
                    view = drh.rearrange("(kc p) o -> p kc o", p=128)
                    for h in range(H):
                        hs = slice(h * 128, (h + 1) * 128)
                        nc.sync.dma_start(out=t[:, :, hs], in_=view[:, :, hs])
                    w_sb[name] = t
                _main(nc, tc, mybir, AF, ADD, AXX, w_sb, xT, outT, idx_sb, mask_sb,
                      ident, relk_sb, xpool, spool, aopool, apool, pjpool,
                      pspoolS, pspoolB, pspoolT, pspoolO, f32, f16)
    nc.compile()
    return nc


def _main(nc, tc, mybir, AF, ADD, AXX, w_sb, xT, outT, idx_sb, mask_sb, ident,
          relk_sb, xpool, spool, aopool, apool, pjpool, pspoolS, pspoolB,
          pspoolT, pspoolO, f32, f16):
    NIT = NREG * H  # 64 head-region iterations, two-stage software pipeline

    # per-iteration live state, keyed it -> dict
    st = {}
    xr_by_reg = {}
    aoT_by_reg = {}

    def stage_front(it):
        """Projections + scores for iteration `it` = (r, h)."""
        r, h = divmod(it, H)
        if h == 0:
            xr = xpool.tile([128, KC, TR + 16], f16, tag="xr")
            xsrc = xT[:, r * TR : r * TR + TR + 16].rearrange(
                "(kc p) n -> p kc n", p=128
            )
            for kc in range(KC):
                nc.sync.dma_start(out=xr[:, kc, :], in_=xsrc[:, kc, :])
            xr_by_reg[r] = xr
        xr = xr_by_reg[r]
        hs = slice(h * 128, (h + 1) * 128)

        # --- projections (fp16 operands, fp32 PSUM accumulate) ---
        qps = pjpool.tile([128, TR + 2], f32, tag="pj")
        kps = pjpool.tile([128, TR + 16], f32, tag="pj")
        vps = pjpool.tile([128, TR + 12], f32, tag="pj")
        for kc in range(KC):
            st_, sp_ = kc == 0, kc == KC - 1
            nc.tensor.matmul(
                qps[:], lhsT=w_sb["wq"][:, kc, hs], rhs=xr[:, kc, 10 : TR + 12],
                start=st_, stop=sp_,
            )
        for kc in range(KC):
            st_, sp_ = kc == 0, kc == KC - 1
            nc.tensor.matmul(
                kps[:], lhsT=w_sb["wk"][:, kc, hs], rhs=xr[:, kc, :],
                start=st_, stop=sp_,
            )
        for kc in range(KC):
            st_, sp_ = kc == 0, kc == KC - 1
            nc.tensor.matmul(
                vps[:], lhsT=w_sb["wv"][:, kc, hs], rhs=xr[:, kc, 0 : TR + 12],
                start=st_, stop=sp_,
            )
        q_sb = spool.tile([128, TR + 2], f16, tag="q")
        k_sb = spool.tile([128, TR + 16], f16, tag="k")
        v_sb = spool.tile([128, TR + 12], f16, tag="v", bufs=3)
        nc.vector.tensor_copy(q_sb[:], qps[:])
        nc.vector.tensor_copy(k_sb[:], kps[:])
        nc.scalar.copy(v_sb[:], vps[:])
        st[it] = dict(q=q_sb, k=k_sb, v=v_sb)

    def stage_scores(it):
        """Content + rel-position score matmuls for iteration `it`."""
        r, h = divmod(it, H)
        s = st[it]
        q_sb, k_sb = s["q"], s["k"]
        sall = pspoolS.tile([TPB, NTILE, WIN], f32, tag="sall")
        bdall = pspoolB.tile([TPB, NTILE, 2 * NP25], f32, tag="bd")
        for g in range(NTILE):
            b0 = TPB * g
            qmain = q_sb[:, b0 + 2 : b0 + 2 + TPB]
            qprev = q_sb[:, b0 + 1 : b0 + 1 + TPB]
            nc.tensor.matmul(
                sall[:, g, :], lhsT=qmain, rhs=k_sb[:, b0 : b0 + WIN],
                start=True, stop=True,
            )
            nc.tensor.matmul(
                bdall[:, g, 0:NP25], lhsT=qmain, rhs=relk_sb[:, h, 0:NP25],
                start=True, stop=True,
            )
            nc.tensor.matmul(
                bdall[:, g, NP25 : 2 * NP25], lhsT=qprev, rhs=relk_sb[:, h, 0:NP25],
                start=True, stop=True,
            )
        s["sall"], s["bdall"] = sall, bdall

    def stage_softmax(it):
        """Scatter + mask + softcap + exp + normalize -> fp16 probs."""
        s = st[it]
        data = apool.tile([TPB, NTILE, 2 * NP25], f16, tag="data")
        nc.vector.tensor_copy(data[:], s["bdall"][:])
        dst = apool.tile([TPB, NTILE * DFREE], f16, tag="dst")
        nc.gpsimd.local_scatter(
            dst[:], data[:], idx_sb[0:TPB, :],
            channels=TPB, num_elems=NTILE * DFREE, num_idxs=NTILE * 2 * NP25,
        )
        nc.gpsimd.tensor_tensor(
            out=dst[:], in0=dst[:], in1=mask_sb[0:TPB, :], op=ADD,
        )
        lg = apool.tile([TPB, NTILE, DFREE], f32, tag="lg")
        nc.vector.tensor_tensor(
            out=lg[:], in0=s["sall"][:],
            in1=dst.rearrange("p (g w) -> p g w", g=NTILE), op=ADD,
        )
        nc.scalar.activation(out=lg[:], in_=lg[:], func=AF.Tanh, scale=1.0 / SOFTCAP)
        nc.scalar.activation(out=lg[:], in_=lg[:], func=AF.Exp, scale=SOFTCAP)
        rsum = apool.tile([TPB, NTILE], f32, tag="rsum")
        nc.vector.tensor_reduce(out=rsum[:], in_=lg[:], axis=AXX, op=ADD)
        nc.vector.reciprocal(rsum[:], rsum[:])
        pr = apool.tile([TPB, NTILE, W], f16, tag="pr", bufs=3)
        for g in range(NTILE):
            nc.gpsimd.tensor_scalar_mul(
                out=pr[:, g, :], in0=lg[:, g, 0:W], scalar1=rsum[:, g : g + 1],
            )
        s["pr"] = pr

    def stage_transpose(it):
        """PE transposes of probs + V for iteration `it` (ready long ago)."""
        s = st[it]
        pr, v_sb = s["pr"], s["v"]
        atvt = pspoolT.tile([W, NTILE, TPB + 128], f16, tag="atvt")
        for g in range(NTILE):
            b0 = TPB * g
            nc.tensor.transpose(
                atvt[:, g, 0:TPB], pr[:, g, :], ident[0:TPB, 0:TPB]
            )
            nc.tensor.transpose(
                atvt[:, g, TPB : TPB + 128], v_sb[:, b0 : b0 + W], ident[:, :]
            )
        s["atvt"] = atvt

    def stage_copies(it):
        """PSUM->SBUF copies of the transposed tiles."""
        s = st[it]
        at_sb = apool.tile([W, NTILE, TPB], f16, tag="at")
        vt_sb = apool.tile([W, NTILE, 128], f16, tag="vt")
        nc.vector.tensor_copy(at_sb[:], s["atvt"][:, :, 0:TPB])
        nc.scalar.copy(vt_sb[:], s["atvt"][:, :, TPB : TPB + 128])
        s["at"], s["vt"] = at_sb, vt_sb

    def stage_out(it):
        """Attention-output matmuls + aoT copy."""
        r, h = divmod(it, H)
        s = st[it]
        aops = pspoolO.tile([128, NTILE, TPB], f32, tag="aops")
        for g in range(NTILE):
            nc.tensor.matmul(
                aops[:, g, :], lhsT=s["vt"][:, g, :], rhs=s["at"][:, g, :],
                start=True, stop=True,
            )
        if h == 0:
            aoT_by_reg[r] = aopool.tile([128, H * TR], f16, tag="aoT", name="aoT")
        nc.vector.tensor_copy(
            aoT_by_reg[r][:, h * TR : (h + 1) * TR], aops[:]
        )
        del st[it]

    def emit_post(r, ocs):
        """Post projection for region r, given output-channel chunks."""
        aoT = aoT_by_reg[r]
        for oc in ocs:
            pps = pjpool.tile([128, TR], f32, tag="pj")
            for h in range(H):
                nc.tensor.matmul(
                    pps[:],
                    lhsT=w_sb["wp"][:, h, oc * 128 : (oc + 1) * 128],
                    rhs=aoT[:, h * TR : (h + 1) * TR],
                    start=(h == 0), stop=(h == H - 1),
                )
            po = apool.tile([128, TR], f32, tag="po")
            if oc % 2 == 0:
                nc.vector.tensor_copy(po[:], pps[:])
            else:
                nc.scalar.copy(po[:], pps[:])
            nc.sync.dma_start(
                out=outT[oc * 128 : (oc + 1) * 128, r * TR : (r + 1) * TR],
                in_=po[:],
            )

    # ---- software-pipelined main loop (2-iteration stagger) ----
    for it in range(NIT + 2):
        fin = it - 2
        if fin >= 0:
            stage_transpose(fin)
            stage_copies(fin)
        if it < NIT:
            stage_front(it)
        if fin >= 0:
            stage_out(fin)
        if it < NIT:
            stage_scores(it)
            stage_softmax(it)
        if it < NIT:
            r, h = divmod(it, H)
            if r > 0 and h in (1, 2, 3, 4):
                emit_post(r - 1, [2 * (h - 1), 2 * (h - 1) + 1])
    emit_post(NREG - 1, list(range(KC)))


def _get_nc():
    if "nc" not in _CACHE:
        _CACHE["nc"] = _build_bass()
    return _CACHE["nc"]


def _prepare_in_maps(hidden_states, position_embeddings, Wq, Wk, Wv, Wpost, Wrel,
                     per_dim_scale):
    f16 = np.float16
    hs = np.asarray(hidden_states, np.float32)
    pe = np.asarray(position_embeddings, np.float32)
    qscale = (Q_SCALE * np.log1p(np.exp(np.asarray(per_dim_scale, np.float64)))).astype(
        np.float64
    )
    qs_tiled = np.tile(qscale, H)  # per output channel o: scale[o % 128]
    wqT = np.ascontiguousarray(
        (np.asarray(Wq, np.float64) * qs_tiled[:, None]).T.astype(f16)
    )
    wkT = np.ascontiguousarray((np.asarray(Wk, np.float64) * K_SCALE).T.astype(f16))
    wvT = np.ascontiguousarray(np.asarray(Wv, np.float32).T.astype(f16))
    wpT = np.ascontiguousarray(np.asarray(Wpost, np.float32).T.astype(f16))
    wrelT = np.ascontiguousarray(np.asarray(Wrel, np.float32).T.astype(f16))
    pembT = np.zeros((HID, 32), f16)
    pembT[:, :P] = pe.T.astype(f16)
    idx, mask = _build_tables()

    shared = dict(wqT=wqT, wkT=wkT, wvT=wvT, wpT=wpT, wrelT=wrelT, pembT=pembT,
                  idxtab=idx, masktab=mask)
    in_maps = []
    for core in range(NCORES):
        b, half = divmod(core, 2)
        lo = half * T
        slab = np.zeros((THALO, HID), np.float32)
        src_lo = max(lo - PAST, 0)
        src_hi = min(lo + T + 4, S)
        off = src_lo - (lo - PAST)
        slab[off : off + (src_hi - src_lo), :] = hs[b, src_lo:src_hi, :]
        xT = np.ascontiguousarray(slab.T.astype(f16))
        in_maps.append(dict(xT=xT, **shared))
    return in_maps


def _assemble(results):
    out = np.empty((B, S, HID), np.float32)
    for core in range(NCORES):
        b, half = divmod(core, 2)
        out[b, half * T : (half + 1) * T, :] = results[core]["outT"].T
    return out


def kernel(**inputs) -> np.ndarray:
    from concourse.bass_utils import run_bass_kernel_spmd

    nc = _get_nc()
    in_maps = _prepare_in_maps(**inputs)
    res = run_bass_kernel_spmd(nc, in_maps, list(range(NCORES)))
    return _assemble(res.results)
